# revision 3
# baseline (speedup 1.0000x reference)
"""GQA per-token attention kernel for Trainium2, 8-core data-parallel.

Reference computation (per token, no cross-token mixing):
  q = x @ Wq.T + bq -> [16 rows of 128]   (rows = (g, kh) flattened)
  k = x @ Wk.T + bk -> [4 heads of 128]
  v = x @ Wv.T + bv -> [4 heads of 128]
  att[r, j] = softmax_j(q_r . k_j / sqrt(128))
  attn_out_r = sum_j att[r, j] * v_j
  y = attn_out @ Wo.T + bo

Sharding: batch*seq = 16384 tokens split contiguously across 8 cores.
Device layout: tokens on SBUF partitions (128/tile); contraction dims on
partitions for matmuls (x pre-transposed on host). Matmuls in bf16 with
fp32 PSUM accumulation; biases folded in as K=1 ones-row matmuls;
per-token attention on DVE/ACT; PE transposes attn_out for the O-proj.
"""

import numpy as np
import ml_dtypes

import concourse.bacc as bacc
import concourse.tile as tile
import concourse.mybir as mybir
from concourse.bass_utils import run_bass_kernel_spmd

N_CORES = 8
HID = 2048
D = 128
HC = HID // D            # 16 hidden chunks
QROWS = 16               # q feature chunks (g * kh)
KVH = 4                  # kv heads
TOK_TOTAL = 16384
TOK_CORE = TOK_TOTAL // N_CORES   # 2048
N_MACRO = 2
TOK_MACRO = TOK_CORE // N_MACRO   # 1024
N_ST = TOK_MACRO // 128           # 8 subtiles per macro

BF = mybir.dt.bfloat16
F32 = mybir.dt.float32
AX = mybir.AxisListType
AF = mybir.ActivationFunctionType
INV_SQRT_D = 1.0 / np.sqrt(128.0)

_CACHED = {}


def _build_nc():
    nc = bacc.Bacc("TRN2", target_bir_lowering=False, num_devices=N_CORES)

    xt_d = nc.dram_tensor("xt", [HC, D, TOK_CORE], BF, kind="ExternalInput")
    wq_d = nc.dram_tensor("wq", [HC, D, HID], BF, kind="ExternalInput")
    wkv_d = nc.dram_tensor("wkv", [HC, D, 1024], BF, kind="ExternalInput")
    wo_d = nc.dram_tensor("wo", [HC, D, HID], BF, kind="ExternalInput")
    bq_d = nc.dram_tensor("bq", [1, HID], BF, kind="ExternalInput")
    bkv_d = nc.dram_tensor("bkv", [1, 1024], BF, kind="ExternalInput")
    bo_d = nc.dram_tensor("bo", [1, HID], BF, kind="ExternalInput")
    id_d = nc.dram_tensor("ident", [D, D], BF, kind="ExternalInput")
    ones_d = nc.dram_tensor("ones", [1, D], BF, kind="ExternalInput")
    y_d = nc.dram_tensor("y", [TOK_CORE, HID], F32, kind="ExternalOutput")

    with tile.TileContext(nc) as tc:
        with (
            tc.tile_pool(name="const", bufs=1) as constp,
            tc.tile_pool(name="wbig", bufs=1) as wbigp,
            tc.tile_pool(name="wkvp", bufs=1) as wkvp,
            tc.tile_pool(name="xtp", bufs=2) as xtp,
            tc.tile_pool(name="qkv", bufs=2) as qkvp,
            tc.tile_pool(name="attnT", bufs=1) as attnp,
            tc.tile_pool(name="av", bufs=3) as avp,
            tc.tile_pool(name="small", bufs=3) as smallp,
            tc.tile_pool(name="ysb", bufs=3) as yp,
            tc.tile_pool(name="mm", bufs=6, space="PSUM") as mmp,
            tc.tile_pool(name="tr", bufs=2, space="PSUM") as trp,
        ):
            ident = constp.tile([D, D], BF, tag="ident")
            nc.sync.dma_start(out=ident[:], in_=id_d[:])
            ones = constp.tile([1, D], BF, tag="ones")
            nc.sync.dma_start(out=ones[:], in_=ones_d[:])
            bq_s = constp.tile([1, HID], BF, tag="bq")
            nc.sync.dma_start(out=bq_s[:], in_=bq_d[:])
            bkv_s = constp.tile([1, 1024], BF, tag="bkv")
            nc.sync.dma_start(out=bkv_s[:], in_=bkv_d[:])
            bo_s = constp.tile([1, HID], BF, tag="bo")
            nc.sync.dma_start(out=bo_s[:], in_=bo_d[:])

            for mac in range(N_MACRO):
                wq = wbigp.tile([D, HC, HID], BF, tag="wbig")
                nc.sync.dma_start(out=wq[:], in_=wq_d.rearrange("c p n -> p c n"))
                wkv = wkvp.tile([D, HC, 1024], BF, tag="wkv")
                nc.sync.dma_start(out=wkv[:], in_=wkv_d.rearrange("c p n -> p c n"))
                attnT = attnp.tile([D, QROWS, TOK_MACRO], BF, tag="attnT")

                for st in range(N_ST):
                    tok0 = mac * TOK_MACRO + st * 128
                    xt = xtp.tile([D, HC, 128], BF, tag="xt")
                    nc.sync.dma_start(
                        out=xt[:],
                        in_=xt_d.rearrange("c p t -> p c t")[:, :, tok0 : tok0 + 128],
                    )

                    # ---- QKV projections: out[tok, of] in PSUM ----
                    q_ps = [mmp.tile([128, 512], F32, tag="mm", name=f"qps{og}")
                            for og in range(4)]
                    k_ps = mmp.tile([128, 512], F32, tag="mm")
                    v_ps = mmp.tile([128, 512], F32, tag="mm")
                    for og in range(4):
                        nc.tensor.matmul(
                            q_ps[og][:], lhsT=ones[:],
                            rhs=bq_s[:, og * 512 : (og + 1) * 512],
                            start=True, stop=False,
                        )
                    nc.tensor.matmul(k_ps[:], lhsT=ones[:], rhs=bkv_s[:, 0:512],
                                     start=True, stop=False)
                    nc.tensor.matmul(v_ps[:], lhsT=ones[:], rhs=bkv_s[:, 512:1024],
                                     start=True, stop=False)
                    for hc in range(HC):
                        lhs = xt[:, hc, :]
                        last = hc == HC - 1
                        for og in range(4):
                            nc.tensor.matmul(
                                q_ps[og][:], lhsT=lhs,
                                rhs=wq[:, hc, og * 512 : (og + 1) * 512],
                                start=False, stop=last,
                            )
                        nc.tensor.matmul(k_ps[:], lhsT=lhs, rhs=wkv[:, hc, 0:512],
                                         start=False, stop=last)
                        nc.tensor.matmul(v_ps[:], lhsT=lhs, rhs=wkv[:, hc, 512:1024],
                                         start=False, stop=last)

                    q_sb = qkvp.tile([128, HID], BF, tag="q")
                    k_sb = qkvp.tile([128, 512], BF, tag="k")
                    v_sb = qkvp.tile([128, 512], BF, tag="v")
                    for og in range(4):
                        nc.scalar.copy(out=q_sb[:, og * 512 : (og + 1) * 512],
                                       in_=q_ps[og][:])
                    nc.scalar.copy(out=k_sb[:], in_=k_ps[:])
                    nc.scalar.copy(out=v_sb[:], in_=v_ps[:])

                    # ---- per-token attention (tokens on partitions) ----
                    q3 = q_sb[:].rearrange("p (g d) -> p g d", g=QROWS)
                    k3 = k_sb[:].rearrange("p (j d) -> p j d", j=KVH)
                    v3 = v_sb[:].rearrange("p (j d) -> p j d", j=KVH)

                    logits = smallp.tile([128, QROWS, KVH], F32, tag="lg")
                    for j in range(KVH):
                        prod = avp.tile([128, QROWS, D], BF, tag="av")
                        nc.vector.tensor_mul(
                            out=prod[:], in0=q3,
                            in1=k3[:, j : j + 1, :].broadcast_to((128, QROWS, D)),
                        )
                        nc.vector.reduce_sum(out=logits[:, :, j], in_=prod[:], axis=AX.X)

                    e = smallp.tile([128, QROWS, KVH], F32, tag="e")
                    nc.scalar.activation(out=e[:], in_=logits[:], func=AF.Exp,
                                         scale=float(INV_SQRT_D))
                    s = smallp.tile([128, QROWS], F32, tag="s")
                    nc.vector.reduce_sum(out=s[:], in_=e[:], axis=AX.X)
                    r = smallp.tile([128, QROWS], F32, tag="r")
                    nc.vector.reciprocal(out=r[:], in_=s[:])
                    att = smallp.tile([128, QROWS, KVH], BF, tag="att")
                    nc.vector.tensor_mul(
                        out=att[:], in0=e[:],
                        in1=r[:, :, None].broadcast_to((128, QROWS, KVH)),
                    )

                    acc = avp.tile([128, QROWS, D], BF, tag="av")
                    nc.vector.tensor_mul(
                        out=acc[:],
                        in0=v3[:, 0:1, :].broadcast_to((128, QROWS, D)),
                        in1=att[:, :, 0:1].broadcast_to((128, QROWS, D)),
                    )
                    for j in range(1, KVH):
                        prod = avp.tile([128, QROWS, D], BF, tag="av")
                        nc.vector.tensor_mul(
                            out=prod[:],
                            in0=v3[:, j : j + 1, :].broadcast_to((128, QROWS, D)),
                            in1=att[:, :, j : j + 1].broadcast_to((128, QROWS, D)),
                        )
                        nc.vector.tensor_add(out=acc[:], in0=acc[:], in1=prod[:])

                    # ---- transpose attn_out -> [of, tok] for the O-proj ----
                    for tg in range(4):
                        tr = trp.tile([128, 4, D], BF, tag="tr")
                        for i in range(4):
                            ofc = tg * 4 + i
                            nc.tensor.transpose(tr[:, i, :], acc[:, ofc, :], ident[:])
                        nc.scalar.copy(
                            out=attnT[:, tg * 4 : (tg + 1) * 4,
                                      st * 128 : (st + 1) * 128],
                            in_=tr[:],
                        )

                # ---- O projection for this macro ----
                wo = wbigp.tile([D, HC, HID], BF, tag="wbig")
                nc.sync.dma_start(out=wo[:], in_=wo_d.rearrange("c p n -> p c n"))
                for st in range(N_ST):
                    tok0 = mac * TOK_MACRO + st * 128
                    y_ps = [mmp.tile([128, 512], F32, tag="mm", name=f"yps{og}")
                            for og in range(4)]
                    for og in range(4):
                        nc.tensor.matmul(
                            y_ps[og][:], lhsT=ones[:],
                            rhs=bo_s[:, og * 512 : (og + 1) * 512],
                            start=True, stop=False,
                        )
                    for ofc in range(QROWS):
                        lhs = attnT[:, ofc, st * 128 : (st + 1) * 128]
                        last = ofc == QROWS - 1
                        for og in range(4):
                            nc.tensor.matmul(
                                y_ps[og][:], lhsT=lhs,
                                rhs=wo[:, ofc, og * 512 : (og + 1) * 512],
                                start=False, stop=last,
                            )
                    for og in range(4):
                        y_sb = yp.tile([128, 512], F32, tag="y")
                        nc.scalar.copy(out=y_sb[:], in_=y_ps[og][:])
                        nc.sync.dma_start(
                            out=y_d[tok0 : tok0 + 128, og * 512 : (og + 1) * 512],
                            in_=y_sb[:],
                        )

    nc.finalize()
    return nc


def _get_nc():
    if "nc" not in _CACHED:
        _CACHED["nc"] = _build_nc()
    return _CACHED["nc"]


def _prep_inputs(x, Wq, bq, Wk, bk, Wv, bv, Wo, bo):
    bf16 = ml_dtypes.bfloat16
    xf = np.ascontiguousarray(x.reshape(TOK_TOTAL, HID))
    shared = {
        "wq": np.ascontiguousarray(Wq.T.reshape(HC, D, HID)).astype(bf16),
        "wkv": np.ascontiguousarray(
            np.concatenate([Wk.T, Wv.T], axis=1).reshape(HC, D, 1024)
        ).astype(bf16),
        "wo": np.ascontiguousarray(Wo.T.reshape(HC, D, HID)).astype(bf16),
        "bq": bq.reshape(1, HID).astype(bf16),
        "bkv": np.concatenate([bk, bv]).reshape(1, 1024).astype(bf16),
        "bo": bo.reshape(1, HID).astype(bf16),
        "ident": np.eye(D, dtype=np.float32).astype(bf16),
        "ones": np.ones((1, D), dtype=np.float32).astype(bf16),
    }
    in_maps = []
    for c in range(N_CORES):
        xs = xf[c * TOK_CORE : (c + 1) * TOK_CORE]
        xt = np.ascontiguousarray(xs.T.reshape(HC, D, TOK_CORE)).astype(bf16)
        in_maps.append({"xt": xt, **shared})
    return in_maps


def kernel(x, Wq, bq, Wk, bk, Wv, bv, Wo, bo):
    x = np.asarray(x, dtype=np.float32)
    nc = _get_nc()
    in_maps = _prep_inputs(np.asarray(x), np.asarray(Wq), np.asarray(bq),
                           np.asarray(Wk), np.asarray(bk), np.asarray(Wv),
                           np.asarray(bv), np.asarray(Wo), np.asarray(bo))
    res = run_bass_kernel_spmd(nc, in_maps, core_ids=list(range(N_CORES)))
    y = np.concatenate([r["y"] for r in res.results], axis=0)
    return y.reshape(x.shape)
